# revision 2
# baseline (speedup 1.0000x reference)
"""TGCN (3-step GRU over GCN message passing) on 8 Trainium2 NeuronCores.

Strategy (per the dst-sharding hint):
- Host relabels nodes (max-pool over nodes is permutation invariant) with a
  degree-balanced LPT assignment into 8 cores x 98 windows x 128 slots.
- Per timestep: phase A (replicated): xi = x @ lin1_w, scaled by dinv =
  rsqrt(deg), written to DRAM as a gather table (4 chunks, fp16).
- Phase B (dst-sharded): edges grouped by (dst window, src chunk); dma_gather
  fetches source rows; a 0/1 selection matrix built with iota+is_equal routes
  each 128-edge block into the window's PSUM accumulator via the PE
  (scatter-add as matmul). Self-loops are explicit edges.
- Phase C: GRU gates as fp16 matmuls feature-major; H stays resident in SBUF.
- Final: per-feature max over the core's nodes, AllReduce-max across cores,
  then the 128x10 output projection (identical on every core).
"""
import sys

sys.path.insert(0, "/opt/trn_rl_repo")

import numpy as np

import concourse.bass as bass
import concourse.mybir as mybir
import concourse.tile as tile
import concourse.bacc as bacc
from concourse.bass import broadcast_tensor_aps
from concourse.bass_utils import run_bass_kernel_spmd
from concourse.masks import make_identity

F16 = mybir.dt.float16
F32 = mybir.dt.float32
I16 = mybir.dt.int16
I32 = mybir.dt.int32

N = 100000
E = 1600000
DIN = 128
DH = 128
DOUT = 10
P = 128
NCORE = 8
NW = 98               # windows (128-slot dst tiles) per core
SPC = NW * P          # 12544 slots per core
NSLOT = NCORE * SPC   # 100352
NT = NSLOT // P       # 784 global tiles
REAL_PC = 12500       # real nodes per core; pads at slots [12500, 12544)
CHN = 4               # source chunks (by window class w % 4)
NWC = [25, 25, 24, 24]            # windows per class (per core)
CHROWS = [NCORE * c * P for c in NWC]   # chunk row counts
CB = 5                # blocks per (window, chunk) cell
CBS = 6               # blocks when chunk == w % 4 (holds self-loop edges)
NBLK_W = 3 * CB + CBS             # 21 blocks per window
WGS = 7               # windows per gather group
NWG = NW // WGS       # 14 groups
TS = 3


def _counts(w, c):
    return CBS if (w % 4) == c else CB


def _nblk_cg(g, c):
    return sum(_counts(w, c) for w in range(g * WGS, (g + 1) * WGS))


def _preprocess(inputs):
    """Numpy-only host prep: node relabeling, edge sharding, input staging."""
    for b in ("lin1_b", "convb_z", "convb_r", "convb_h",
              "linb_z", "linb_r", "linb_h", "lin2_b"):
        assert np.abs(np.asarray(inputs[b])).max() == 0.0, f"{b} nonzero"

    import heapq

    edges = [np.asarray(inputs[f"edge{t}"]).astype(np.int64) for t in range(TS)]
    deg3 = np.zeros(N, np.int64)
    for t in range(TS):
        deg3 += np.bincount(edges[t][1], minlength=N)
    w_nodes = deg3 + 3

    order = np.argsort(-w_nodes, kind="stable")
    nbins = NCORE * NW
    cap = np.full(nbins, P, np.int32)
    cap[NW - 1 :: NW] = REAL_PC - (NW - 1) * P  # 84 real slots in last window
    heap = [(0, b) for b in range(nbins)]
    heapq.heapify(heap)
    bin_count = np.zeros(nbins, np.int32)
    bin_load = np.zeros(nbins, np.int64)
    assign_bin = np.empty(N, np.int32)
    slot_in_bin = np.empty(N, np.int32)
    for n in order:
        load, b = heapq.heappop(heap)
        assign_bin[n] = b
        slot_in_bin[n] = bin_count[b]
        bin_count[b] += 1
        bin_load[b] += w_nodes[n]
        if bin_count[b] < cap[b]:
            heapq.heappush(heap, (bin_load[b], b))
    core_of = assign_bin // NW
    w_of = assign_bin % NW
    gslot = (core_of * SPC + w_of * P + slot_in_bin).astype(np.int64)

    # x staged in permuted slot order (fp16), pads zero
    x_perm = np.zeros((TS, NSLOT, DIN), np.float16)
    for t in range(TS):
        x_perm[t, gslot] = np.asarray(inputs[f"x{t}"]).astype(np.float16)

    # degrees (with +1 self loop); pads get 1.0
    deg_all = np.ones((TS, P, NT), np.float32)
    deg_my = np.ones((NCORE, TS, P, NW), np.float32)
    for t in range(TS):
        dd = np.bincount(gslot[edges[t][1]], minlength=NSLOT).astype(np.float32)
        dd[gslot] += 1.0  # self loops for real slots; pads stay at the init 1.0
        dd2 = dd.copy()
        dd2[dd2 == 0] = 1.0
        # mark pads (no self loop added) as 1.0: real slots had +1 so >=1
        deg_all[t] = dd2.reshape(NT, P).T
        for k in range(NCORE):
            deg_my[k, t] = dd2[k * SPC : (k + 1) * SPC].reshape(NW, P).T

    # chunk-local row index of a global slot
    wcls = (np.arange(NSLOT) % SPC) // P % 4
    corearr = np.arange(NSLOT) // SPC
    warr = (np.arange(NSLOT) % SPC) // P
    parr = np.arange(NSLOT) % P
    nwc_arr = np.array(NWC)
    srcloc_of = (corearr * nwc_arr[wcls] * P + (warr // 4) * P + parr).astype(np.int64)

    max_cols = max(_nblk_cg(g, c) for g in range(NWG) for c in range(CHN)) * P // 16
    idx_arr = np.zeros((NCORE, TS, CHN, NWG, 16, max_cols), np.int16)
    dst_arr = np.full((NCORE, TS, NWG, P, WGS * NBLK_W), -1.0, np.float16)

    CAPC = CBS * P  # padded cell capacity used during fill
    for t in range(TS):
        src, dst = edges[t]
        gs = np.concatenate([gslot[src], gslot])  # + self loops
        gd = np.concatenate([gslot[dst], gslot])
        kcore = gd // SPC
        w = (gd % SPC) // P
        dstrel = gd % P
        ws = (gs % SPC) // P
        ch = ws % 4
        srcloc = srcloc_of[gs]
        key = ((kcore * NW + w) * CHN + ch).astype(np.int64)
        o = np.argsort(key, kind="stable")
        key_s, srcloc_s, dstrel_s = key[o], srcloc[o], dstrel[o]
        ncell = NCORE * NW * CHN
        cnt = np.bincount(key_s, minlength=ncell)
        starts = np.concatenate([[0], np.cumsum(cnt)[:-1]])
        rank = np.arange(len(key_s)) - starts[key_s]
        capv = np.where(
            (np.arange(ncell) % CHN) == ((np.arange(ncell) // CHN) % NW) % 4, CBS, CB
        ) * P
        assert (cnt <= capv).all(), (cnt.max(), "cell overflow: raise CB/CBS")
        pad_src = np.zeros((NCORE, NW, CHN, CAPC), np.int64)
        pad_dst = np.full((NCORE, NW, CHN, CAPC), -1.0, np.float32)
        flat_cell = key_s
        pad_src.reshape(ncell, CAPC)[flat_cell, rank] = srcloc_s
        pad_dst.reshape(ncell, CAPC)[flat_cell, rank] = dstrel_s
        for k in range(NCORE):
            for g in range(NWG):
                for c in range(CHN):
                    parts = [
                        pad_src[k, w2, c, : _counts(w2, c) * P]
                        for w2 in range(g * WGS, (g + 1) * WGS)
                    ]
                    flat = np.concatenate(parts)
                    assert flat.max() < 32768
                    a16 = flat.astype(np.int16).reshape(-1, 16).T  # [16, n/16]
                    idx_arr[k, t, c, g, :, : a16.shape[1]] = a16
                for wl in range(WGS):
                    w2 = g * WGS + wl
                    blocks = []
                    for c in range(CHN):
                        nb = _counts(w2, c)
                        blocks.append(pad_dst[k, w2, c, : nb * P].reshape(nb, P))
                    bl = np.concatenate(blocks, axis=0)  # [21, 128]
                    dst_arr[k, t, g, :, wl * NBLK_W : (wl + 1) * NBLK_W] = (
                        bl.T.astype(np.float16)
                    )

    wts = dict(
        lin1_w=np.asarray(inputs["lin1_w"]).astype(np.float16),
        lin2_w=np.asarray(inputs["lin2_w"]).astype(np.float32),
    )
    for gname in "zrh":
        wts[f"convW_{gname}"] = np.asarray(inputs[f"convW_{gname}"]).astype(np.float16)
        lw = np.asarray(inputs[f"linW_{gname}"]).astype(np.float16)
        wts[f"linWt_{gname}"] = lw[:DH]
        wts[f"linWb_{gname}"] = lw[DH:]

    idx_arr = np.ascontiguousarray(np.tile(idx_arr, (1, 1, 1, 1, 8, 1)))  # [.., 128, cols]
    return dict(
        x_perm=x_perm, deg_all=deg_all, deg_my=deg_my,
        idx_arr=idx_arr, dst_arr=dst_arr, wts=wts,
    )


def _build(phases="ABCF", reps=1, ndev=NCORE):
    nc = bacc.Bacc("TRN2", target_bir_lowering=False, debug=False, num_devices=ndev)

    max_cols = max(_nblk_cg(g, c) for g in range(NWG) for c in range(CHN)) * P // 16
    x_in = nc.dram_tensor("x_perm", [TS, NSLOT, DIN], F16, kind="ExternalInput")
    degall_in = nc.dram_tensor("deg_all", [TS, P, NT], F32, kind="ExternalInput")
    degmy_in = nc.dram_tensor("deg_my", [TS, P, NW], F32, kind="ExternalInput")
    idx_in = nc.dram_tensor("idx_arr", [TS, CHN, NWG, P, max_cols], I16,
                            kind="ExternalInput")
    dst_in = nc.dram_tensor("dst_arr", [TS, NWG, P, WGS * NBLK_W], F16,
                            kind="ExternalInput")
    lin1_in = nc.dram_tensor("lin1_w", [DIN, DH], F16, kind="ExternalInput")
    convW_in = {g: nc.dram_tensor(f"convW_{g}", [DH, DH], F16, kind="ExternalInput")
                for g in "zrh"}
    linWt_in = {g: nc.dram_tensor(f"linWt_{g}", [DH, DH], F16, kind="ExternalInput")
                for g in "zrh"}
    linWb_in = {g: nc.dram_tensor(f"linWb_{g}", [DH, DH], F16, kind="ExternalInput")
                for g in "zrh"}
    lin2_in = nc.dram_tensor("lin2_w", [DH, DOUT], F32, kind="ExternalInput")
    out_t = nc.dram_tensor("out", [1, DOUT], F32, kind="ExternalOutput")

    xs_c = [nc.dram_tensor(f"xs_c{c}", [CHROWS[c], DH], F16) for c in range(CHN)]

    with tile.TileContext(nc) as tc:
        with (
            tc.tile_pool(name="const", bufs=1) as cpool,
            tc.tile_pool(name="hpool", bufs=1) as hpool,
            tc.tile_pool(name="pa", bufs=3) as pa,          # phase A sbuf
            tc.tile_pool(name="gb", bufs=2) as gb,          # gather bufs
            tc.tile_pool(name="bc", bufs=3) as bcp,         # phase B/C small tiles
            tc.tile_pool(name="ps", bufs=8, space="PSUM") as ps,
            tc.tile_pool(name="dram", bufs=1, space="DRAM") as dr,
        ):
            # constants
            lin1_sb = cpool.tile([DIN, DH], F16, tag="w")
            nc.sync.dma_start(lin1_sb[:], lin1_in[:])
            convW_sb = {}
            linWt_sb = {}
            linWb_sb = {}
            for g in "zrh":
                convW_sb[g] = cpool.tile([DH, DH], F16, tag=f"cw{g}", name=f"cw{g}")
                nc.sync.dma_start(convW_sb[g][:], convW_in[g][:])
                linWt_sb[g] = cpool.tile([DH, DH], F16, tag=f"lt{g}", name=f"lt{g}")
                nc.sync.dma_start(linWt_sb[g][:], linWt_in[g][:])
                linWb_sb[g] = cpool.tile([DH, DH], F16, tag=f"lb{g}", name=f"lb{g}")
                nc.sync.dma_start(linWb_sb[g][:], linWb_in[g][:])
            lin2_sb = cpool.tile([DH, 16], F32, tag="l2")
            nc.gpsimd.memset(lin2_sb[:], 0.0)
            nc.sync.dma_start(lin2_sb[:, :DOUT], lin2_in[:])

            iota_i = cpool.tile([P, P], I32, tag="ioi")
            nc.gpsimd.iota(iota_i[:], pattern=[[1, P]], base=0, channel_multiplier=0)
            iota_f = cpool.tile([P, P], F16, tag="iof")
            nc.vector.tensor_copy(iota_f[:], iota_i[:])
            ident = cpool.tile([P, P], F16, tag="id")
            make_identity(nc, ident[:])

            H_sb = hpool.tile([DH, SPC], F16, tag="H")
            nc.gpsimd.memset(H_sb[:], 0.0)

            dinv_all = cpool.tile([P, NT], F32, tag="dia")
            dinv_my = cpool.tile([P, NW], F32, tag="dim")

            for t_i in range(TS * reps):
                t = t_i % TS
                # ---- dinv for this timestep ----
                dtmp = pa.tile([P, NT], F32, tag="dtmp")
                nc.sync.dma_start(dtmp[:], degall_in[t])
                nc.vector.reciprocal(dtmp[:], dtmp[:])
                nc.scalar.sqrt(dinv_all[:], dtmp[:])
                dtmp2 = pa.tile([P, NW], F32, tag="dtmp2")
                nc.sync.dma_start(dtmp2[:], degmy_in[t])
                nc.vector.reciprocal(dtmp2[:], dtmp2[:])
                nc.scalar.sqrt(dinv_my[:], dtmp2[:])

                # ---- phase A: xs = dinv * (x @ lin1_w), all 784 tiles ----
                for grp in range(NT // 4 if "A" in phases else 0):
                    xi_ps = ps.tile([P, 512], F32, tag="ps")
                    xT = [None] * 4
                    for b in range(4):
                        T = grp * 4 + b
                        xT[b] = pa.tile([P, P], F16, tag="xT", name="xT")
                        nc.sync.dma_start(
                            xT[b][:], x_in[t, T * P : (T + 1) * P, :], transpose=True
                        )
                        nc.tensor.matmul(
                            xi_ps[:, b * P : (b + 1) * P],
                            lhsT=xT[b][:],
                            rhs=lin1_sb[:],
                            start=True,
                            stop=True,
                        )
                    xs_sb = pa.tile([P, 512], F16, tag="xs")
                    i0 = xi_ps[:].rearrange("p (b q) -> p b q", b=4)
                    i1 = dinv_all[:, grp * 4 : grp * 4 + 4][:, :, None]
                    a0, a1 = broadcast_tensor_aps(i0, i1)
                    o3 = xs_sb[:].rearrange("p (b q) -> p b q", b=4)
                    nc.vector.tensor_tensor(out=o3, in0=a0, in1=a1,
                                            op=mybir.AluOpType.mult)
                    for b in range(4):
                        T = grp * 4 + b
                        core, w = T // NW, T % NW
                        c = w % 4
                        row = core * NWC[c] * P + (w // 4) * P
                        nc.sync.dma_start(
                            xs_c[c][row : row + P, :], xs_sb[:, b * P : (b + 1) * P]
                        )

                # ---- phase B + C per gather group ----
                for g in range(NWG if ("B" in phases or "G" in phases) else 0):
                    Gt = [None] * CHN
                    for c in range(CHN):
                        nblk = _nblk_cg(g, c)
                        ncols = nblk * P // 16
                        ix = gb.tile([P, max_cols], I16, tag=f"ix{c}")
                        nc.sync.dma_start(ix[:, :ncols], idx_in[t, c, g, :, :ncols])
                        Gt[c] = gb.tile([P, 37 * P], F16, tag=f"G{c}", name=f"G{c}")
                        g3 = Gt[c][:, : nblk * P].rearrange("p (b q) -> p b q", q=P)
                        nc.gpsimd.dma_gather(
                            g3,
                            xs_c[c][:],
                            ix[:, :ncols],
                            num_idxs=nblk * P,
                            num_idxs_reg=nblk * P,
                            elem_size=P,
                            single_packet=False,
                        )
                    dst_sb = gb.tile([P, WGS * NBLK_W], F16, tag="dst")
                    nc.sync.dma_start(dst_sb[:], dst_in[t, g])
                    if "B" not in phases:
                        continue

                    goff = [0] * CHN
                    for wl in range(WGS):
                        w = g * WGS + wl
                        # selection matrices for all 21 blocks in one op
                        M01 = bcp.tile([P, NBLK_W * P], F16, tag="m01")
                        m3 = M01[:].rearrange("p (b q) -> p b q", b=NBLK_W)
                        i0 = iota_f[:].rearrange("p (b q) -> p b q", b=1)
                        i1 = dst_sb[:, wl * NBLK_W : (wl + 1) * NBLK_W][:, :, None]
                        a0, a1 = broadcast_tensor_aps(i0, i1)
                        nc.vector.tensor_tensor(out=m3, in0=a0, in1=a1,
                                                op=mybir.AluOpType.is_equal)
                        S_ps = ps.tile([P, P], F32, tag="ps")
                        blk = 0
                        for c in range(CHN):
                            nb = _counts(w, c)
                            for b in range(nb):
                                nc.tensor.matmul(
                                    S_ps[:],
                                    lhsT=M01[:, (blk) * P : (blk + 1) * P],
                                    rhs=Gt[c][:, (goff[c] + b) * P : (goff[c] + b + 1) * P],
                                    start=(blk == 0),
                                    stop=(blk == NBLK_W - 1),
                                )
                                blk += 1
                            goff[c] += nb
                        # Y = dinv_dst * S   (node-major [dst, fo])
                        Y_sb = bcp.tile([P, P], F16, tag="Y")
                        nc.vector.tensor_scalar(
                            out=Y_sb[:], in0=S_ps[:],
                            scalar1=dinv_my[:, w : w + 1], scalar2=None,
                            op0=mybir.AluOpType.mult,
                        )
                        if "C" not in phases:
                            continue
                        # transpose Y -> feature-major
                        Yt_ps = ps.tile([P, P], F16, tag="ps")
                        nc.tensor.transpose(Yt_ps[:], Y_sb[:], ident[:])
                        Yt_sb = bcp.tile([P, P], F16, tag="Yt")
                        nc.scalar.activation(Yt_sb[:], Yt_ps[:],
                                             mybir.ActivationFunctionType.Copy)
                        # conv per gate
                        Q_sb = {}
                        for gi, gname in enumerate("zrh"):
                            Q_ps = ps.tile([P, P], F32, tag="ps")
                            nc.tensor.matmul(Q_ps[:], lhsT=convW_sb[gname][:],
                                             rhs=Yt_sb[:], start=True, stop=True)
                            Q_sb[gname] = bcp.tile([P, P], F16, tag=f"Q{gname}", name=f"Q{gname}")
                            if gi % 2 == 0:
                                nc.vector.tensor_copy(Q_sb[gname][:], Q_ps[:])
                            else:
                                nc.scalar.activation(
                                    Q_sb[gname][:], Q_ps[:],
                                    mybir.ActivationFunctionType.Copy)
                        Hsl = H_sb[:, w * P : (w + 1) * P]
                        # z and r gates
                        ZR = {}
                        for gname in "zr":
                            A_ps = ps.tile([P, P], F32, tag="ps")
                            nc.tensor.matmul(A_ps[:], lhsT=linWt_sb[gname][:],
                                             rhs=Q_sb[gname][:], start=True, stop=False)
                            nc.tensor.matmul(A_ps[:], lhsT=linWb_sb[gname][:],
                                             rhs=Hsl, start=False, stop=True)
                            ZR[gname] = bcp.tile([P, P], F16, tag=gname.upper(), name=gname.upper())
                            nc.scalar.activation(ZR[gname][:], A_ps[:],
                                                 mybir.ActivationFunctionType.Sigmoid)
                        HR = bcp.tile([P, P], F16, tag="HR")
                        nc.vector.tensor_mul(HR[:], Hsl, ZR["r"][:])
                        A_ps = ps.tile([P, P], F32, tag="ps")
                        nc.tensor.matmul(A_ps[:], lhsT=linWt_sb["h"][:],
                                         rhs=Q_sb["h"][:], start=True, stop=False)
                        nc.tensor.matmul(A_ps[:], lhsT=linWb_sb["h"][:],
                                         rhs=HR[:], start=False, stop=True)
                        Ht = bcp.tile([P, P], F16, tag="Ht")
                        nc.scalar.activation(Ht[:], A_ps[:],
                                             mybir.ActivationFunctionType.Tanh)
                        # H = Ht + Z*(H - Ht)
                        Hd = bcp.tile([P, P], F16, tag="Hd")
                        nc.vector.tensor_sub(Hd[:], Hsl, Ht[:])
                        nc.vector.tensor_mul(Hd[:], ZR["z"][:], Hd[:])
                        nc.vector.tensor_add(Hsl, Ht[:], Hd[:])

            # ---- final: masked max pool + AllReduce + projection ----
            nc.gpsimd.memset(H_sb[:, REAL_PC:SPC], -10000.0)
            hmax = cpool.tile([P, 1], F32, tag="hmax")
            nc.vector.reduce_max(hmax[:], H_sb[:], axis=mybir.AxisListType.X)
            cc_in = dr.tile([P, 1], F32)
            cc_out = dr.tile([P, 1], F32)
            nc.sync.dma_start(cc_in[:], hmax[:])
            if ndev > 1:
                nc.gpsimd.collective_compute(
                    "AllReduce",
                    mybir.AluOpType.max,
                    replica_groups=[list(range(NCORE))],
                    ins=[cc_in.opt()],
                    outs=[cc_out.opt()],
                )
            else:
                nc.gpsimd.dma_start(cc_out[:], cc_in[:])
            hg = cpool.tile([P, 1], F32, tag="hg")
            nc.sync.dma_start(hg[:], cc_out[:])
            o_ps = ps.tile([1, 16], F32, tag="ps")
            nc.tensor.matmul(o_ps[:, :16], lhsT=hg[:], rhs=lin2_sb[:],
                             start=True, stop=True)
            o_sb = cpool.tile([1, 16], F32, tag="osb")
            nc.vector.tensor_copy(o_sb[:], o_ps[:])
            nc.sync.dma_start(out_t[:], o_sb[:, :DOUT])

    nc.compile()
    return nc


def _make_in_maps(pre):
    in_maps = []
    for k in range(NCORE):
        in_maps.append(
            dict(
                x_perm=pre["x_perm"],
                deg_all=pre["deg_all"],
                deg_my=np.ascontiguousarray(pre["deg_my"][k]),
                idx_arr=np.ascontiguousarray(pre["idx_arr"][k]),
                dst_arr=np.ascontiguousarray(pre["dst_arr"][k]),
                lin1_w=pre["wts"]["lin1_w"],
                lin2_w=pre["wts"]["lin2_w"],
                **{f"convW_{g}": pre["wts"][f"convW_{g}"] for g in "zrh"},
                **{f"linWt_{g}": pre["wts"][f"linWt_{g}"] for g in "zrh"},
                **{f"linWb_{g}": pre["wts"][f"linWb_{g}"] for g in "zrh"},
            )
        )
    return in_maps


def _postprocess(res, pre):
    return res.results[0]["out"].astype(np.float32)


def kernel(**inputs) -> np.ndarray:
    pre = _preprocess(inputs)
    nc = _build()
    in_maps = _make_in_maps(pre)
    res = run_bass_kernel_spmd(nc, in_maps, core_ids=list(range(NCORE)))
    return _postprocess(res, pre)


if __name__ == "__main__":
    d = dict(np.load("/root/problem/inputs_cache.npz"))
    out = kernel(**d)
    print("kernel out:", out)



# revision 9
# speedup vs baseline: 2.9294x; 2.9294x over previous
"""TGCN (3-step GRU over GCN message passing) on 8 Trainium2 NeuronCores.

Strategy (dst-sharded message passing):
- Host relabels nodes (max-pool over nodes is permutation invariant) with a
  degree-balanced LPT assignment into 8 cores x 98 windows x 128 slots.
- Per timestep, phase A (replicated on every core): xs = dinv * (x @ lin1_w)
  written to DRAM as a node-major gather table in global tile order, split
  into 4 contiguous chunks of 25088 rows (int16-indexable). x is staged
  feature-major by the host so phase A needs no DMA transposes; each group
  of 4 tiles is one 128KB contiguous read + one 128KB contiguous write.
- Phase B (dst-sharded): edges grouped into (7-window group, src chunk)
  gather calls; dma_gather (rotating over 4 SWDGE queues so descriptor
  rings drain in parallel) fetches the source rows; 0/1 selection matrices
  built with iota+is_equal route each 128-edge block into the window's
  PSUM accumulator via the PE (scatter-add as matmul). Self-loops skip the
  gather entirely: a small per-core copy of the core's own x columns is
  run through lin1 again and added with an identity matmul.
- Phase C: GRU gates as fp16 matmuls feature-major; H stays resident in SBUF.
- xs tables double-buffered across timesteps so phase A of step t+1 overlaps
  the gathers/GRU of step t.
- Final: per-feature max over the core's nodes, AllReduce-max across cores,
  then the 128x10 output projection (identical on every core).
"""
import sys

sys.path.insert(0, "/opt/trn_rl_repo")

import numpy as np

import concourse.bass as bass
import concourse.mybir as mybir
import concourse.tile as tile
import concourse.bacc as bacc
from concourse.bass import broadcast_tensor_aps
from concourse.bass_utils import run_bass_kernel_spmd
from concourse.masks import make_identity

F16 = mybir.dt.float16
F32 = mybir.dt.float32
I16 = mybir.dt.int16
I32 = mybir.dt.int32

N = 100000
E = 1600000
DIN = 128
DH = 128
DOUT = 10
P = 128
NCORE = 8
NW = 98               # windows (128-slot dst tiles) per core
SPC = NW * P          # 12544 slots per core
NSLOT = NCORE * SPC   # 100352
NT = NSLOT // P       # 784 global tiles
REAL_PC = 12500       # real nodes per core; pads at slots [12500, 12544)
CHN = 4               # source chunks: contiguous tile ranges of 196 tiles
CHTILES = NT // CHN   # 196
CHSZ = CHTILES * P    # 25088 rows per chunk (< 32768: int16-safe)
WGS = 7               # windows per gather group
NWG = NW // WGS       # 14 groups
TS = 3
AGRP = 196            # phase A groups of 4 tiles (49 per chunk)


def _preprocess(inputs):
    """Numpy-only host prep: node relabeling, edge sharding, input staging."""
    for b in ("lin1_b", "convb_z", "convb_r", "convb_h",
              "linb_z", "linb_r", "linb_h", "lin2_b"):
        assert np.abs(np.asarray(inputs[b])).max() == 0.0, f"{b} nonzero"

    import heapq

    edges = [np.asarray(inputs[f"edge{t}"]).astype(np.int64) for t in range(TS)]
    deg3 = np.zeros(N, np.int64)
    for t in range(TS):
        deg3 += np.bincount(edges[t][1], minlength=N)
    w_nodes = deg3 + 3

    order = np.argsort(-w_nodes, kind="stable")
    nbins = NCORE * NW
    cap = np.full(nbins, P, np.int32)
    cap[NW - 1 :: NW] = REAL_PC - (NW - 1) * P  # 84 real slots in last window
    heap = [(0, b) for b in range(nbins)]
    heapq.heapify(heap)
    bin_count = np.zeros(nbins, np.int32)
    bin_load = np.zeros(nbins, np.int64)
    assign_bin = np.empty(N, np.int32)
    slot_in_bin = np.empty(N, np.int32)
    for n in order:
        load, b = heapq.heappop(heap)
        assign_bin[n] = b
        slot_in_bin[n] = bin_count[b]
        bin_count[b] += 1
        bin_load[b] += w_nodes[n]
        if bin_count[b] < cap[b]:
            heapq.heappush(heap, (bin_load[b], b))
    core_of = assign_bin // NW
    w_of = assign_bin % NW
    gslot = (core_of * SPC + w_of * P + slot_in_bin).astype(np.int64)

    # x staged feature-major (transposed) in permuted slot order; pads zero
    xT = np.zeros((TS, NSLOT, DIN), np.float16)
    for t in range(TS):
        xT[t, gslot] = np.asarray(inputs[f"x{t}"]).astype(np.float16)
    xT = np.ascontiguousarray(xT.transpose(0, 2, 1))  # [TS, DIN, NSLOT]

    # per-core replica of the core's own x columns (for the self-loop path)
    xT_self = np.empty((NCORE, TS, DIN, SPC), np.float16)
    for k in range(NCORE):
        xT_self[k] = xT[:, :, k * SPC : (k + 1) * SPC]

    # degrees (with +1 self loop); pads get 1.0 -> dinv arrays
    dinv_all = np.empty((TS, P, NT), np.float32)
    dinv_my = np.empty((NCORE, TS, P, NW), np.float32)
    for t in range(TS):
        dd = np.bincount(gslot[edges[t][1]], minlength=NSLOT).astype(np.float64)
        dd += 1.0  # self loops (pads harmlessly get deg 1: their xs rows are 0)
        di = (1.0 / np.sqrt(dd)).astype(np.float32)
        dinv_all[t] = di.reshape(NT, P).T
        for k in range(NCORE):
            dinv_my[k, t] = di[k * SPC : (k + 1) * SPC].reshape(NW, P).T

    # ---- edge cells: (core, window, chunk), capacity CB blocks each ----
    cellcnt_max = 0
    percell = []
    for t in range(TS):
        src, dst = edges[t]
        gs, gd = gslot[src], gslot[dst]
        key = (gd // P) * CHN + gs // CHSZ  # (core*NW + w) * CHN + chunk
        cnt = np.bincount(key, minlength=NCORE * NW * CHN)
        percell.append((gs, gd, key))
        cellcnt_max = max(cellcnt_max, int(cnt.max()))
    CB = max(5, -(-cellcnt_max // P))  # blocks per cell (uniform across cores)
    NBLK_W = CHN * CB                 # blocks per window
    CALL_BLK = WGS * CB               # blocks per (group, chunk) call
    NIDX_CALL = CALL_BLK * P
    CAPC = CB * P

    idx_arr = np.zeros((NCORE, TS, CHN, NWG, 16, NIDX_CALL // 16), np.int16)
    dst_arr = np.full((NCORE, TS, NWG, P, WGS * NBLK_W), -1.0, np.float16)

    ncell = NCORE * NW * CHN
    for t in range(TS):
        gs, gd, key = percell[t]
        srcloc = (gs % CHSZ).astype(np.int64)
        dstrel = gd % P
        o = np.argsort(key, kind="stable")
        key_s, srcloc_s, dstrel_s = key[o], srcloc[o], dstrel[o]
        cnt = np.bincount(key_s, minlength=ncell)
        starts = np.concatenate([[0], np.cumsum(cnt)[:-1]])
        rank = np.arange(len(key_s)) - starts[key_s]
        pad_src = np.zeros((ncell, CAPC), np.int64)
        pad_dst = np.full((ncell, CAPC), -1.0, np.float32)
        pad_src[key_s, rank] = srcloc_s
        pad_dst[key_s, rank] = dstrel_s
        pad_src = pad_src.reshape(NCORE, NW, CHN, CAPC)
        pad_dst = pad_dst.reshape(NCORE, NW, CHN, CAPC)
        for k in range(NCORE):
            for g in range(NWG):
                ws = slice(g * WGS, (g + 1) * WGS)
                for c in range(CHN):
                    flat = pad_src[k, ws, c].reshape(-1)  # [WGS*CAPC]
                    assert flat.max() < CHSZ
                    idx_arr[k, t, c, g] = (
                        flat.astype(np.int16).reshape(-1, 16).T
                    )
                # dst columns: (wl*NBLK_W + c*CB + b)
                d4 = pad_dst[k, ws].reshape(WGS, CHN * CB, P)  # [7, 20, 128]
                dst_arr[k, t, g] = (
                    d4.reshape(WGS * NBLK_W, P).T.astype(np.float16)
                )

    idx_arr = np.ascontiguousarray(np.tile(idx_arr, (1, 1, 1, 1, 8, 1)))

    wts = dict(
        lin1_w=np.asarray(inputs["lin1_w"]).astype(np.float16),
        lin2_w=np.asarray(inputs["lin2_w"]).astype(np.float32),
    )
    for gname in "zrh":
        wts[f"convW_{gname}"] = np.asarray(inputs[f"convW_{gname}"]).astype(np.float16)
        lw = np.asarray(inputs[f"linW_{gname}"]).astype(np.float16)
        wts[f"linWt_{gname}"] = lw[:DH]
        wts[f"linWb_{gname}"] = lw[DH:]

    return dict(
        xT=xT, xT_self=xT_self, dinv_all=dinv_all, dinv_my=dinv_my,
        idx_arr=idx_arr, dst_arr=dst_arr, wts=wts, CB=CB,
    )


def _build(CB=5, ndev=NCORE):
    NBLK_W = CHN * CB
    CALL_BLK = WGS * CB
    NIDX_CALL = CALL_BLK * P
    ICOLS = NIDX_CALL // 16

    nc = bacc.Bacc("TRN2", target_bir_lowering=False, debug=False,
                   num_devices=ndev, num_swdge_queues=4)

    xT_in = nc.dram_tensor("xT", [TS, DIN, NSLOT], F16, kind="ExternalInput")
    xTs_in = nc.dram_tensor("xT_self", [TS, DIN, SPC], F16, kind="ExternalInput")
    dia_in = nc.dram_tensor("dinv_all", [TS, P, NT], F32, kind="ExternalInput")
    dim_in = nc.dram_tensor("dinv_my", [TS, P, NW], F32, kind="ExternalInput")
    idx_in = nc.dram_tensor("idx_arr", [TS, CHN, NWG, P, ICOLS], I16,
                            kind="ExternalInput")
    dst_in = nc.dram_tensor("dst_arr", [TS, NWG, P, WGS * NBLK_W], F16,
                            kind="ExternalInput")
    lin1_in = nc.dram_tensor("lin1_w", [DIN, DH], F16, kind="ExternalInput")
    convW_in = {g: nc.dram_tensor(f"convW_{g}", [DH, DH], F16, kind="ExternalInput")
                for g in "zrh"}
    linWt_in = {g: nc.dram_tensor(f"linWt_{g}", [DH, DH], F16, kind="ExternalInput")
                for g in "zrh"}
    linWb_in = {g: nc.dram_tensor(f"linWb_{g}", [DH, DH], F16, kind="ExternalInput")
                for g in "zrh"}
    lin2_in = nc.dram_tensor("lin2_w", [DH, DOUT], F32, kind="ExternalInput")
    out_t = nc.dram_tensor("out", [1, DOUT], F32, kind="ExternalOutput")

    # xs gather tables, double-buffered across timesteps
    xs_d = [[nc.dram_tensor(f"xs_p{pr}c{c}", [CHSZ, DH], F16) for c in range(CHN)]
            for pr in range(2)]

    with tile.TileContext(nc) as tc:
        with (
            tc.tile_pool(name="const", bufs=1) as cpool,
            tc.tile_pool(name="hpool", bufs=1) as hpool,
            tc.tile_pool(name="pa", bufs=3) as pa,          # phase A sbuf
            tc.tile_pool(name="gb", bufs=2) as gb,          # gather bufs
            tc.tile_pool(name="bc", bufs=3) as bcp,         # phase B/C small tiles
            tc.tile_pool(name="ps", bufs=1, space="PSUM") as ps,
            tc.tile_pool(name="dram", bufs=1, space="DRAM") as dr,
        ):
            # constants
            lin1_sb = cpool.tile([DIN, DH], F16, tag="w")
            nc.sync.dma_start(lin1_sb[:], lin1_in[:])
            convW_sb = {}
            linWt_sb = {}
            linWb_sb = {}
            for g in "zrh":
                convW_sb[g] = cpool.tile([DH, DH], F16, tag=f"cw{g}", name=f"cw{g}")
                nc.sync.dma_start(convW_sb[g][:], convW_in[g][:])
                linWt_sb[g] = cpool.tile([DH, DH], F16, tag=f"lt{g}", name=f"lt{g}")
                nc.sync.dma_start(linWt_sb[g][:], linWt_in[g][:])
                linWb_sb[g] = cpool.tile([DH, DH], F16, tag=f"lb{g}", name=f"lb{g}")
                nc.sync.dma_start(linWb_sb[g][:], linWb_in[g][:])
            lin2_sb = cpool.tile([DH, 16], F32, tag="l2")
            nc.gpsimd.memset(lin2_sb[:], 0.0)
            nc.sync.dma_start(lin2_sb[:, :DOUT], lin2_in[:])

            iota_i = cpool.tile([P, P], I32, tag="ioi")
            nc.gpsimd.iota(iota_i[:], pattern=[[1, P]], base=0, channel_multiplier=0)
            iota_f = cpool.tile([P, P], F16, tag="iof")
            nc.vector.tensor_copy(iota_f[:], iota_i[:])
            ident = cpool.tile([P, P], F16, tag="id")
            make_identity(nc, ident[:])

            H_sb = hpool.tile([DH, SPC], F16, tag="H")
            nc.gpsimd.memset(H_sb[:], 0.0)

            for t in range(TS):
                par = t % 2
                # ---- per-timestep dinv tables ----
                dinv_all = pa.tile([P, NT], F32, tag="dia")
                nc.sync.dma_start(dinv_all[:], dia_in[t])
                dinv_my = pa.tile([P, NW], F32, tag="dim")
                nc.sync.dma_start(dinv_my[:], dim_in[t])

                # ---- phase A: xs = dinv * (x @ lin1_w) ----
                for grp in range(AGRP):
                    ch = grp // 49
                    row0 = (grp % 49) * 512
                    xT_sb = pa.tile([P, 512], F16, tag="xT", name="xT")
                    nc.sync.dma_start(
                        xT_sb[:], xT_in[t, :, grp * 512 : (grp + 1) * 512]
                    )
                    xi_ps = ps.tile([P, 512], F32, tag="xi", bufs=2)
                    for b in range(4):
                        nc.tensor.matmul(
                            xi_ps[:, b * P : (b + 1) * P],
                            lhsT=xT_sb[:, b * P : (b + 1) * P],
                            rhs=lin1_sb[:],
                            start=True,
                            stop=True,
                        )
                    xs_sb = pa.tile([P, 512], F16, tag="xs")
                    i0 = xi_ps[:].rearrange("p (b q) -> p b q", b=4)
                    i1 = dinv_all[:, grp * 4 : grp * 4 + 4][:, :, None]
                    a0, a1 = broadcast_tensor_aps(i0, i1)
                    o3 = xs_sb[:].rearrange("p (b q) -> p b q", b=4)
                    nc.vector.tensor_tensor(out=o3, in0=a0, in1=a1,
                                            op=mybir.AluOpType.mult)
                    tgt = xs_d[par][ch][row0 : row0 + 512, :].rearrange(
                        "(b p) f -> p b f", p=P
                    )
                    nc.sync.dma_start(
                        tgt, xs_sb[:].rearrange("p (b f) -> p b f", b=4)
                    )

                # ---- phase B + C per gather group ----
                for g in range(NWG):
                    Gt = [None] * CHN
                    for c in range(CHN):
                        ix = gb.tile([P, ICOLS], I16, tag=f"ix{c}")
                        nc.sync.dma_start(ix[:], idx_in[t, c, g])
                        Gt[c] = gb.tile([P, CALL_BLK * P], F16, tag=f"G{c}",
                                        name=f"G{c}")
                        g3 = Gt[c][:].rearrange("p (b q) -> p b q", q=P)
                        nc.gpsimd.dma_gather(
                            g3,
                            xs_d[par][c][:],
                            ix[:],
                            num_idxs=NIDX_CALL,
                            num_idxs_reg=NIDX_CALL,
                            elem_size=P,
                            single_packet=False,
                            queue_num=c,
                        )
                    dst_sb = gb.tile([P, WGS * NBLK_W], F16, tag="dst")
                    nc.sync.dma_start(dst_sb[:], dst_in[t, g])

                    y_ps = [
                        ps.tile([P, 512], F32, tag="Y", name="Y0", bufs=3),
                        ps.tile([P, 512], F32, tag="Y", name="Y1", bufs=3),
                    ]
                    for wl in range(WGS):
                        w = g * WGS + wl
                        ycol = y_ps[wl // 4][:, (wl % 4) * P : (wl % 4 + 1) * P]
                        # selection matrices for this window's blocks in one op
                        M01 = bcp.tile([P, NBLK_W * P], F16, tag="m01")
                        m3 = M01[:].rearrange("p (b q) -> p b q", b=NBLK_W)
                        i0 = iota_f[:].rearrange("p (b q) -> p b q", b=1)
                        i1 = dst_sb[:, wl * NBLK_W : (wl + 1) * NBLK_W][:, :, None]
                        a0, a1 = broadcast_tensor_aps(i0, i1)
                        nc.vector.tensor_tensor(out=m3, in0=a0, in1=a1,
                                                op=mybir.AluOpType.is_equal)
                        # self-loop: recompute xs for this window's own nodes
                        xw_sb = bcp.tile([P, P], F16, tag="xw")
                        nc.sync.dma_start(xw_sb[:], xTs_in[t, :, w * P : (w + 1) * P])
                        xw_ps = ps.tile([P, P], F32, tag="pc", name="xwps", bufs=3)
                        nc.tensor.matmul(xw_ps[:], lhsT=xw_sb[:], rhs=lin1_sb[:],
                                         start=True, stop=True)
                        xsw_sb = bcp.tile([P, P], F16, tag="xsw")
                        nc.vector.tensor_scalar(
                            out=xsw_sb[:], in0=xw_ps[:],
                            scalar1=dinv_my[:, w : w + 1], scalar2=None,
                            op0=mybir.AluOpType.mult,
                        )
                        for c in range(CHN):
                            for b in range(CB):
                                nc.tensor.matmul(
                                    ycol,
                                    lhsT=M01[:, (c * CB + b) * P : (c * CB + b + 1) * P],
                                    rhs=Gt[c][:, (wl * CB + b) * P : (wl * CB + b + 1) * P],
                                    start=(c == 0 and b == 0),
                                    stop=False,
                                )
                        nc.tensor.matmul(ycol, lhsT=ident[:], rhs=xsw_sb[:],
                                         start=False, stop=True)
                        # Y = dinv_dst * S   (node-major [dst, f])
                        Y_sb = bcp.tile([P, P], F16, tag="Yn")
                        nc.vector.tensor_scalar(
                            out=Y_sb[:], in0=ycol,
                            scalar1=dinv_my[:, w : w + 1], scalar2=None,
                            op0=mybir.AluOpType.mult,
                        )
                        # transpose Y -> feature-major
                        Yt_ps = ps.tile([P, P], F16, tag="pc", name="Ytps", bufs=3)
                        nc.tensor.transpose(Yt_ps[:], Y_sb[:], ident[:])
                        Yt_sb = bcp.tile([P, P], F16, tag="Yt")
                        nc.scalar.activation(Yt_sb[:], Yt_ps[:],
                                             mybir.ActivationFunctionType.Copy)
                        # conv per gate
                        Q_sb = {}
                        for gi, gname in enumerate("zrh"):
                            Q_ps = ps.tile([P, P], F32, tag="pc", name=f"Qps{gname}", bufs=3)
                            nc.tensor.matmul(Q_ps[:], lhsT=convW_sb[gname][:],
                                             rhs=Yt_sb[:], start=True, stop=True)
                            Q_sb[gname] = bcp.tile([P, P], F16, tag=f"Q{gname}",
                                                   name=f"Q{gname}")
                            if gi % 2 == 0:
                                nc.vector.tensor_copy(Q_sb[gname][:], Q_ps[:])
                            else:
                                nc.scalar.activation(
                                    Q_sb[gname][:], Q_ps[:],
                                    mybir.ActivationFunctionType.Copy)
                        Hsl = H_sb[:, w * P : (w + 1) * P]
                        # z and r gates
                        ZR = {}
                        for gname in "zr":
                            A_ps = ps.tile([P, P], F32, tag="pc", name=f"Aps{gname}", bufs=3)
                            nc.tensor.matmul(A_ps[:], lhsT=linWt_sb[gname][:],
                                             rhs=Q_sb[gname][:], start=True, stop=False)
                            nc.tensor.matmul(A_ps[:], lhsT=linWb_sb[gname][:],
                                             rhs=Hsl, start=False, stop=True)
                            ZR[gname] = bcp.tile([P, P], F16, tag=gname.upper(),
                                                 name=gname.upper())
                            nc.scalar.activation(ZR[gname][:], A_ps[:],
                                                 mybir.ActivationFunctionType.Sigmoid)
                        HR = bcp.tile([P, P], F16, tag="HR")
                        nc.vector.tensor_mul(HR[:], Hsl, ZR["r"][:])
                        A_ps = ps.tile([P, P], F32, tag="pc", name="Apsh", bufs=3)
                        nc.tensor.matmul(A_ps[:], lhsT=linWt_sb["h"][:],
                                         rhs=Q_sb["h"][:], start=True, stop=False)
                        nc.tensor.matmul(A_ps[:], lhsT=linWb_sb["h"][:],
                                         rhs=HR[:], start=False, stop=True)
                        Ht = bcp.tile([P, P], F16, tag="Ht")
                        nc.scalar.activation(Ht[:], A_ps[:],
                                             mybir.ActivationFunctionType.Tanh)
                        # H = Ht + Z*(H - Ht)
                        Hd = bcp.tile([P, P], F16, tag="Hd")
                        nc.vector.tensor_sub(Hd[:], Hsl, Ht[:])
                        nc.vector.tensor_mul(Hd[:], ZR["z"][:], Hd[:])
                        nc.vector.tensor_add(Hsl, Ht[:], Hd[:])

            # ---- final: masked max pool + AllReduce + projection ----
            nc.gpsimd.memset(H_sb[:, REAL_PC:SPC], -10000.0)
            hmax = cpool.tile([P, 1], F32, tag="hmax")
            nc.vector.reduce_max(hmax[:], H_sb[:], axis=mybir.AxisListType.X)
            cc_in = dr.tile([P, 1], F32)
            cc_out = dr.tile([P, 1], F32)
            nc.sync.dma_start(cc_in[:], hmax[:])
            if ndev > 1:
                nc.gpsimd.collective_compute(
                    "AllReduce",
                    mybir.AluOpType.max,
                    replica_groups=[list(range(NCORE))],
                    ins=[cc_in.opt()],
                    outs=[cc_out.opt()],
                )
            else:
                nc.gpsimd.dma_start(cc_out[:], cc_in[:])
            hg = cpool.tile([P, 1], F32, tag="hg")
            nc.sync.dma_start(hg[:], cc_out[:])
            o_ps = ps.tile([1, 16], F32, tag="pc", bufs=3)
            nc.tensor.matmul(o_ps[:, :16], lhsT=hg[:], rhs=lin2_sb[:],
                             start=True, stop=True)
            o_sb = cpool.tile([1, 16], F32, tag="osb")
            nc.vector.tensor_copy(o_sb[:], o_ps[:])
            nc.sync.dma_start(out_t[:], o_sb[:, :DOUT])

    nc.compile()
    return nc


def _make_in_maps(pre):
    in_maps = []
    for k in range(NCORE):
        in_maps.append(
            dict(
                xT=pre["xT"],
                xT_self=np.ascontiguousarray(pre["xT_self"][k]),
                dinv_all=pre["dinv_all"],
                dinv_my=np.ascontiguousarray(pre["dinv_my"][k]),
                idx_arr=np.ascontiguousarray(pre["idx_arr"][k]),
                dst_arr=np.ascontiguousarray(pre["dst_arr"][k]),
                lin1_w=pre["wts"]["lin1_w"],
                lin2_w=pre["wts"]["lin2_w"],
                **{f"convW_{g}": pre["wts"][f"convW_{g}"] for g in "zrh"},
                **{f"linWt_{g}": pre["wts"][f"linWt_{g}"] for g in "zrh"},
                **{f"linWb_{g}": pre["wts"][f"linWb_{g}"] for g in "zrh"},
            )
        )
    return in_maps


def _postprocess(res, pre):
    return res.results[0]["out"].astype(np.float32)


def kernel(**inputs) -> np.ndarray:
    pre = _preprocess(inputs)
    nc = _build(CB=pre["CB"])
    in_maps = _make_in_maps(pre)
    res = run_bass_kernel_spmd(nc, in_maps, core_ids=list(range(NCORE)))
    return _postprocess(res, pre)


if __name__ == "__main__":
    d = dict(np.load("/root/problem/inputs_cache.npz"))
    out = kernel(**d)
    print("kernel out:", out)


# revision 21
# speedup vs baseline: 3.4607x; 1.1814x over previous
"""TGCN (3-step GRU over GCN message passing) on 8 Trainium2 NeuronCores.

Strategy (dst-sharded message passing):
- Host relabels nodes (max-pool over nodes is permutation invariant) with a
  degree-balanced LPT assignment into 8 cores x 98 windows x 128 slots.
- Per timestep, phase A (replicated on every core): xs = dinv * (x @ lin1_w)
  written to DRAM as a node-major gather table in global tile order, split
  into 4 contiguous chunks of 25088 rows (int16-indexable). x is staged
  feature-major by the host so phase A needs no DMA transposes; each group
  of 4 tiles is one 128KB contiguous read + one 128KB contiguous write.
- Phase B (dst-sharded): edges grouped into (7-window group, src chunk)
  gather calls; dma_gather (rotating over 4 SWDGE queues so descriptor
  rings drain in parallel) fetches the source rows; 0/1 selection matrices
  built with iota+is_equal route each 128-edge block into the window's
  PSUM accumulator via the PE (scatter-add as matmul). Self-loops skip the
  gather entirely: a small per-core copy of the core's own x columns is
  run through lin1 again and added with an identity matmul.
- Phase C: GRU gates as fp16 matmuls feature-major; H stays resident in SBUF.
- xs tables double-buffered across timesteps so phase A of step t+1 overlaps
  the gathers/GRU of step t.
- Final: per-feature max over the core's nodes, AllReduce-max across cores,
  then the 128x10 output projection (identical on every core).
"""
import sys

sys.path.insert(0, "/opt/trn_rl_repo")

import numpy as np

import concourse.bass as bass
import concourse.mybir as mybir
import concourse.tile as tile
import concourse.bacc as bacc
from concourse.bass import broadcast_tensor_aps
from concourse.bass_utils import run_bass_kernel_spmd
from concourse.masks import make_identity

F16 = mybir.dt.float16
F32 = mybir.dt.float32
I16 = mybir.dt.int16
I32 = mybir.dt.int32

N = 100000
E = 1600000
DIN = 128
DH = 128
DOUT = 10
P = 128
NCORE = 8
NW = 98               # windows (128-slot dst tiles) per core
SPC = NW * P          # 12544 slots per core
NSLOT = NCORE * SPC   # 100352
NT = NSLOT // P       # 784 global tiles
REAL_PC = 12500       # real nodes per core; pads at slots [12500, 12544)
CHN = 4               # source chunks: contiguous tile ranges of 196 tiles
CHTILES = NT // CHN   # 196
CHSZ = CHTILES * P    # 25088 rows per chunk (< 32768: int16-safe)
WGS = 7               # windows per gather group
NWG = NW // WGS       # 14 groups
TS = 3
AGRP = 196            # phase A groups of 4 tiles (49 per chunk)


def _preprocess(inputs):
    """Numpy-only host prep: node relabeling, edge sharding, input staging."""
    for b in ("lin1_b", "convb_z", "convb_r", "convb_h",
              "linb_z", "linb_r", "linb_h", "lin2_b"):
        assert np.abs(np.asarray(inputs[b])).max() == 0.0, f"{b} nonzero"

    import heapq

    edges = [np.asarray(inputs[f"edge{t}"]).astype(np.int64) for t in range(TS)]
    deg3 = np.zeros(N, np.int64)
    for t in range(TS):
        deg3 += np.bincount(edges[t][1], minlength=N)
    w_nodes = deg3 + 3

    order = np.argsort(-w_nodes, kind="stable")
    nbins = NCORE * NW
    cap = np.full(nbins, P, np.int32)
    cap[NW - 1 :: NW] = REAL_PC - (NW - 1) * P  # 84 real slots in last window
    heap = [(0, b) for b in range(nbins)]
    heapq.heapify(heap)
    bin_count = np.zeros(nbins, np.int32)
    bin_load = np.zeros(nbins, np.int64)
    assign_bin = np.empty(N, np.int32)
    slot_in_bin = np.empty(N, np.int32)
    for n in order:
        load, b = heapq.heappop(heap)
        assign_bin[n] = b
        slot_in_bin[n] = bin_count[b]
        bin_count[b] += 1
        bin_load[b] += w_nodes[n]
        if bin_count[b] < cap[b]:
            heapq.heappush(heap, (bin_load[b], b))
    core_of = assign_bin // NW
    w_of = assign_bin % NW
    gslot = (core_of * SPC + w_of * P + slot_in_bin).astype(np.int64)

    # x staged feature-major (transposed) in permuted slot order; pads zero
    xT = np.zeros((TS, NSLOT, DIN), np.float16)
    for t in range(TS):
        xT[t, gslot] = np.asarray(inputs[f"x{t}"]).astype(np.float16)
    xT = np.ascontiguousarray(xT.transpose(0, 2, 1))  # [TS, DIN, NSLOT]

    # per-core replica of the core's own x columns (for the self-loop path)
    xT_self = np.empty((NCORE, TS, DIN, SPC), np.float16)
    for k in range(NCORE):
        xT_self[k] = xT[:, :, k * SPC : (k + 1) * SPC]

    # degrees (with +1 self loop); pads get 1.0 -> dinv arrays
    dinv_all = np.empty((TS, P, NT), np.float32)
    dinv_myT = np.empty((NCORE, TS, 1, SPC), np.float32)
    for t in range(TS):
        dd = np.bincount(gslot[edges[t][1]], minlength=NSLOT).astype(np.float64)
        dd += 1.0  # self loops (pads harmlessly get deg 1: their xs rows are 0)
        di = (1.0 / np.sqrt(dd)).astype(np.float32)
        dinv_all[t] = di.reshape(NT, P).T
        for k in range(NCORE):
            dinv_myT[k, t, 0] = di[k * SPC : (k + 1) * SPC]
    # replicated across partitions (DVE can't broadcast along partitions)
    dinv_myT = np.ascontiguousarray(np.broadcast_to(dinv_myT, (NCORE, TS, P, SPC)))

    # ---- edge cells: (core, window, chunk), capacity CB blocks each ----
    cellcnt_max = 0
    percell = []
    for t in range(TS):
        src, dst = edges[t]
        gs, gd = gslot[src], gslot[dst]
        key = (gd // P) * CHN + gs // CHSZ  # (core*NW + w) * CHN + chunk
        cnt = np.bincount(key, minlength=NCORE * NW * CHN)
        percell.append((gs, gd, key))
        cellcnt_max = max(cellcnt_max, int(cnt.max()))
    CB = max(5, -(-cellcnt_max // P))  # blocks per cell (uniform across cores)
    NBLK_W = CHN * CB                 # blocks per window
    CALL_BLK = WGS * CB               # blocks per (group, chunk) call
    NIDX_CALL = CALL_BLK * P
    CAPC = CB * P

    idx_arr = np.zeros((NCORE, TS, CHN, NWG, 16, NIDX_CALL // 16), np.int16)
    dst_arr = np.full((NCORE, TS, NWG, P, WGS * NBLK_W), -1.0, np.float16)

    ncell = NCORE * NW * CHN
    for t in range(TS):
        gs, gd, key = percell[t]
        srcloc = (gs % CHSZ).astype(np.int64)
        dstrel = gd % P
        o = np.argsort(key, kind="stable")
        key_s, srcloc_s, dstrel_s = key[o], srcloc[o], dstrel[o]
        cnt = np.bincount(key_s, minlength=ncell)
        starts = np.concatenate([[0], np.cumsum(cnt)[:-1]])
        rank = np.arange(len(key_s)) - starts[key_s]
        pad_src = np.zeros((ncell, CAPC), np.int64)
        pad_dst = np.full((ncell, CAPC), -1.0, np.float32)
        pad_src[key_s, rank] = srcloc_s
        pad_dst[key_s, rank] = dstrel_s
        pad_src = pad_src.reshape(NCORE, NW, CHN, CAPC)
        pad_dst = pad_dst.reshape(NCORE, NW, CHN, CAPC)
        for k in range(NCORE):
            for g in range(NWG):
                ws = slice(g * WGS, (g + 1) * WGS)
                for c in range(CHN):
                    flat = pad_src[k, ws, c].reshape(-1)  # [WGS*CAPC]
                    assert flat.max() < CHSZ
                    idx_arr[k, t, c, g] = (
                        flat.astype(np.int16).reshape(-1, 16).T
                    )
                # dst columns: (wl*NBLK_W + c*CB + b)
                d4 = pad_dst[k, ws].reshape(WGS, CHN * CB, P)  # [7, 20, 128]
                dst_arr[k, t, g] = (
                    d4.reshape(WGS * NBLK_W, P).T.astype(np.float16)
                )

    idx_arr = np.ascontiguousarray(np.tile(idx_arr, (1, 1, 1, 1, 8, 1)))

    wts = dict(
        lin1_w=np.asarray(inputs["lin1_w"]).astype(np.float16),
        lin2_w=np.asarray(inputs["lin2_w"]).astype(np.float32),
    )
    for gname in "zrh":
        cw = np.asarray(inputs[f"convW_{gname}"]).astype(np.float32)
        lw = np.asarray(inputs[f"linW_{gname}"]).astype(np.float32)
        # fuse conv into the gate projection: Y @ convW @ linW_top
        wts[f"Wc_{gname}"] = (cw @ lw[:DH]).astype(np.float16)
        wts[f"linWb_{gname}"] = lw[DH:].astype(np.float16)

    return dict(
        xT=xT, xT_self=xT_self, dinv_all=dinv_all, dinv_myT=dinv_myT,
        idx_arr=idx_arr, dst_arr=dst_arr, wts=wts, CB=CB,
    )


def _build(CB=5, ndev=NCORE):
    NBLK_W = CHN * CB
    CALL_BLK = WGS * CB
    NIDX_CALL = CALL_BLK * P
    ICOLS = NIDX_CALL // 16

    nc = bacc.Bacc("TRN2", target_bir_lowering=False, debug=False,
                   num_devices=ndev, num_swdge_queues=4)

    xT_in = nc.dram_tensor("xT", [TS, DIN, NSLOT], F16, kind="ExternalInput")
    xTs_in = nc.dram_tensor("xT_self", [TS, DIN, SPC], F16, kind="ExternalInput")
    dia_in = nc.dram_tensor("dinv_all", [TS, P, NT], F32, kind="ExternalInput")
    dim_in = nc.dram_tensor("dinv_myT", [TS, P, SPC], F32, kind="ExternalInput")
    idx_in = nc.dram_tensor("idx_arr", [TS, CHN, NWG, P, ICOLS], I16,
                            kind="ExternalInput")
    dst_in = nc.dram_tensor("dst_arr", [TS, NWG, P, WGS * NBLK_W], F16,
                            kind="ExternalInput")
    lin1_in = nc.dram_tensor("lin1_w", [DIN, DH], F16, kind="ExternalInput")
    Wc_in = {g: nc.dram_tensor(f"Wc_{g}", [DH, DH], F16, kind="ExternalInput")
             for g in "zrh"}
    linWb_in = {g: nc.dram_tensor(f"linWb_{g}", [DH, DH], F16, kind="ExternalInput")
                for g in "zrh"}
    lin2_in = nc.dram_tensor("lin2_w", [DH, DOUT], F32, kind="ExternalInput")
    out_t = nc.dram_tensor("out", [1, DOUT], F32, kind="ExternalOutput")

    # xs gather tables, double-buffered across timesteps
    xs_d = [[nc.dram_tensor(f"xs_p{pr}c{c}", [CHSZ, DH], F16) for c in range(CHN)]
            for pr in range(2)]

    with tile.TileContext(nc) as tc:
        with (
            tc.tile_pool(name="const", bufs=1) as cpool,
            tc.tile_pool(name="hpool", bufs=1) as hpool,
            tc.tile_pool(name="pa", bufs=3) as pa,          # phase A sbuf
            tc.tile_pool(name="gb", bufs=2) as gb,          # gather bufs
            tc.tile_pool(name="bc", bufs=3) as bcp,         # phase B/C small tiles
            tc.tile_pool(name="ps", bufs=1, space="PSUM") as ps,
            tc.tile_pool(name="dram", bufs=1, space="DRAM") as dr,
        ):
            # constants
            lin1_sb = cpool.tile([DIN, DH], F16, tag="w")
            nc.sync.dma_start(lin1_sb[:], lin1_in[:])
            Wc_sb = {}
            linWb_sb = {}
            for g in "zrh":
                Wc_sb[g] = cpool.tile([DH, DH], F16, tag=f"wc{g}", name=f"wc{g}")
                nc.sync.dma_start(Wc_sb[g][:], Wc_in[g][:])
                linWb_sb[g] = cpool.tile([DH, DH], F16, tag=f"lb{g}", name=f"lb{g}")
                nc.sync.dma_start(linWb_sb[g][:], linWb_in[g][:])
            lin2_sb = cpool.tile([DH, 16], F32, tag="l2")
            nc.gpsimd.memset(lin2_sb[:], 0.0)
            nc.sync.dma_start(lin2_sb[:, :DOUT], lin2_in[:])

            iota_i = cpool.tile([P, P], I32, tag="ioi")
            nc.gpsimd.iota(iota_i[:], pattern=[[1, P]], base=0, channel_multiplier=0)
            iota_f = cpool.tile([P, P], F16, tag="iof")
            nc.vector.tensor_copy(iota_f[:], iota_i[:])

            H_sb = hpool.tile([DH, SPC], F16, tag="H")
            nc.gpsimd.memset(H_sb[:], 0.0)

            for t in range(TS):
                par = t % 2
                # ---- per-timestep dinv tables ----
                dinv_all = pa.tile([P, NT], F32, tag="dia")
                nc.sync.dma_start(dinv_all[:], dia_in[t])


                # ---- phase A: xs = dinv * (x @ lin1_w) ----
                for grp in range(AGRP):
                    ch = grp // 49
                    row0 = (grp % 49) * 512
                    xT_sb = pa.tile([P, 512], F16, tag="xT", name="xT")
                    nc.sync.dma_start(
                        xT_sb[:], xT_in[t, :, grp * 512 : (grp + 1) * 512]
                    )
                    xi_ps = ps.tile([P, 512], F32, tag="xi", bufs=2)
                    for b in range(4):
                        nc.tensor.matmul(
                            xi_ps[:, b * P : (b + 1) * P],
                            lhsT=xT_sb[:, b * P : (b + 1) * P],
                            rhs=lin1_sb[:],
                            start=True,
                            stop=True,
                        )
                    xs_sb = pa.tile([P, 512], F16, tag="xs")
                    i0 = xi_ps[:].rearrange("p (b q) -> p b q", b=4)
                    i1 = dinv_all[:, grp * 4 : grp * 4 + 4][:, :, None]
                    a0, a1 = broadcast_tensor_aps(i0, i1)
                    o3 = xs_sb[:].rearrange("p (b q) -> p b q", b=4)
                    nc.vector.tensor_tensor(out=o3, in0=a0, in1=a1,
                                            op=mybir.AluOpType.mult)
                    tgt = xs_d[par][ch][row0 : row0 + 512, :].rearrange(
                        "(b p) f -> p b f", p=P
                    )
                    nc.sync.dma_start(
                        tgt, xs_sb[:].rearrange("p (b f) -> p b f", b=4)
                    )

                # ---- phase B + C per gather group ----
                for g in range(NWG):
                    Gt = [None] * CHN
                    for c in range(CHN):
                        ix = gb.tile([P, ICOLS], I16, tag=f"ix{c}")
                        nc.sync.dma_start(ix[:], idx_in[t, c, g])
                        Gt[c] = gb.tile([P, CALL_BLK * P], F16, tag=f"G{c}",
                                        name=f"G{c}")
                        g3 = Gt[c][:].rearrange("p (b q) -> p b q", q=P)
                        nc.gpsimd.dma_gather(
                            g3,
                            xs_d[par][c][:],
                            ix[:],
                            num_idxs=NIDX_CALL,
                            num_idxs_reg=NIDX_CALL,
                            elem_size=P,
                            single_packet=False,
                            queue_num=c,
                        )
                    dst_sb = gb.tile([P, WGS * NBLK_W], F16, tag="dst")
                    nc.sync.dma_start(dst_sb[:], dst_in[t, g])
                    dinv_g = pa.tile([P, WGS * P], F32, tag="dim", bufs=2)
                    nc.sync.dma_start(
                        dinv_g[:], dim_in[t, :, g * WGS * P : (g + 1) * WGS * P]
                    )

                    y_ps = [
                        ps.tile([P, 512], F32, tag="Y", name="Y0", bufs=3),
                        ps.tile([P, 512], F32, tag="Y", name="Y1", bufs=3),
                    ]
                    for wl in range(WGS):
                        w = g * WGS + wl
                        ycol = y_ps[wl // 4][:, (wl % 4) * P : (wl % 4 + 1) * P]
                        # selection matrices for this window's blocks in one op
                        M01 = bcp.tile([P, NBLK_W * P], F16, tag="m01")
                        m3 = M01[:].rearrange("p (b q) -> p b q", b=NBLK_W)
                        i0 = iota_f[:].rearrange("p (b q) -> p b q", b=1)
                        i1 = dst_sb[:, wl * NBLK_W : (wl + 1) * NBLK_W][:, :, None]
                        a0, a1 = broadcast_tensor_aps(i0, i1)
                        nc.vector.tensor_tensor(out=m3, in0=a0, in1=a1,
                                                op=mybir.AluOpType.is_equal)
                        # aggregate feature-major: Y^T[f, dst] += G^T M01
                        for c in range(CHN):
                            for b in range(CB):
                                nc.tensor.matmul(
                                    ycol,
                                    lhsT=Gt[c][:, (wl * CB + b) * P : (wl * CB + b + 1) * P],
                                    rhs=M01[:, (c * CB + b) * P : (c * CB + b + 1) * P],
                                    start=(c == 0 and b == 0),
                                    stop=(c == CHN - 1 and b == CB - 1),
                                )
                    # ---- phase C in 2 batches: windows [0:4) and [4:7) ----
                    for bi, (w0, nwb) in enumerate(((0, 4), (4, 3))):
                        W = nwb * P
                        n0 = (g * WGS + w0) * P  # node-column base
                        nsl = slice(n0, n0 + W)
                        Hsl = H_sb[:, nsl]
                        drow = dinv_g[:, w0 * P : w0 * P + W]
                        # self-loop xs recompute (feature-major, one matmul)
                        xw_sb = bcp.tile([P, 512], F16, tag="xw")
                        nc.sync.dma_start(xw_sb[:, :W], xTs_in[t, :, nsl])
                        xi_ps = ps.tile([P, 512], F32, tag="pc", name="xisf", bufs=3)
                        nc.tensor.matmul(xi_ps[:, :W], lhsT=lin1_sb[:],
                                         rhs=xw_sb[:, :W], start=True, stop=True)
                        xsw_sb = bcp.tile([P, 512], F32, tag="xsw")
                        nc.vector.tensor_tensor(out=xsw_sb[:, :W], in0=xi_ps[:, :W],
                                                in1=drow, op=mybir.AluOpType.mult)
                        # Yt = (S + xs_self) * dinv_dst   (feature-major)
                        y0_sb = bcp.tile([P, 512], F32, tag="y0")
                        nc.vector.tensor_add(y0_sb[:, :W], y_ps[bi][:, :W],
                                             xsw_sb[:, :W])
                        Yt_sb = bcp.tile([P, 512], F16, tag="Yt")
                        nc.vector.tensor_tensor(out=Yt_sb[:, :W], in0=y0_sb[:, :W],
                                                in1=drow, op=mybir.AluOpType.mult)
                        # gates (conv fused into Wc on host)
                        ZR = {}
                        for gname in "zr":
                            A_ps = ps.tile([P, 512], F32, tag="pc",
                                           name=f"Aps{gname}", bufs=3)
                            nc.tensor.matmul(A_ps[:, :W], lhsT=Wc_sb[gname][:],
                                             rhs=Yt_sb[:, :W], start=True, stop=False)
                            nc.tensor.matmul(A_ps[:, :W], lhsT=linWb_sb[gname][:],
                                             rhs=Hsl, start=False, stop=True)
                            ZR[gname] = bcp.tile([P, 512], F16, tag=gname.upper(),
                                                 name=gname.upper())
                            nc.scalar.activation(ZR[gname][:, :W], A_ps[:, :W],
                                                 mybir.ActivationFunctionType.Sigmoid)
                        HR = bcp.tile([P, 512], F16, tag="HR")
                        nc.vector.tensor_mul(HR[:, :W], Hsl, ZR["r"][:, :W])
                        A_ps = ps.tile([P, 512], F32, tag="pc", name="Apsh", bufs=3)
                        nc.tensor.matmul(A_ps[:, :W], lhsT=Wc_sb["h"][:],
                                         rhs=Yt_sb[:, :W], start=True, stop=False)
                        nc.tensor.matmul(A_ps[:, :W], lhsT=linWb_sb["h"][:],
                                         rhs=HR[:, :W], start=False, stop=True)
                        Ht = bcp.tile([P, 512], F16, tag="Ht")
                        nc.scalar.activation(Ht[:, :W], A_ps[:, :W],
                                             mybir.ActivationFunctionType.Tanh)
                        # H = Ht + Z*(H - Ht)
                        Hd = bcp.tile([P, 512], F16, tag="Hd")
                        nc.vector.tensor_sub(Hd[:, :W], Hsl, Ht[:, :W])
                        nc.vector.tensor_mul(Hd[:, :W], ZR["z"][:, :W], Hd[:, :W])
                        nc.vector.tensor_add(Hsl, Ht[:, :W], Hd[:, :W])

            # ---- final: masked max pool + AllReduce + projection ----
            nc.gpsimd.memset(H_sb[:, REAL_PC:SPC], -10000.0)
            hmax = cpool.tile([P, 1], F32, tag="hmax")
            nc.vector.reduce_max(hmax[:], H_sb[:], axis=mybir.AxisListType.X)
            cc_in = dr.tile([P, 1], F32)
            cc_out = dr.tile([P, 1], F32)
            nc.sync.dma_start(cc_in[:], hmax[:])
            if ndev > 1:
                nc.gpsimd.collective_compute(
                    "AllReduce",
                    mybir.AluOpType.max,
                    replica_groups=[list(range(NCORE))],
                    ins=[cc_in.opt()],
                    outs=[cc_out.opt()],
                )
            else:
                nc.gpsimd.dma_start(cc_out[:], cc_in[:])
            hg = cpool.tile([P, 1], F32, tag="hg")
            nc.sync.dma_start(hg[:], cc_out[:])
            o_ps = ps.tile([1, 16], F32, tag="pc", bufs=3)
            nc.tensor.matmul(o_ps[:, :16], lhsT=hg[:], rhs=lin2_sb[:],
                             start=True, stop=True)
            o_sb = cpool.tile([1, 16], F32, tag="osb")
            nc.vector.tensor_copy(o_sb[:], o_ps[:])
            nc.sync.dma_start(out_t[:], o_sb[:, :DOUT])

    nc.compile()
    return nc


def _make_in_maps(pre):
    in_maps = []
    for k in range(NCORE):
        in_maps.append(
            dict(
                xT=pre["xT"],
                xT_self=np.ascontiguousarray(pre["xT_self"][k]),
                dinv_all=pre["dinv_all"],
                dinv_myT=np.ascontiguousarray(pre["dinv_myT"][k]),
                idx_arr=np.ascontiguousarray(pre["idx_arr"][k]),
                dst_arr=np.ascontiguousarray(pre["dst_arr"][k]),
                lin1_w=pre["wts"]["lin1_w"],
                lin2_w=pre["wts"]["lin2_w"],
                **{f"Wc_{g}": pre["wts"][f"Wc_{g}"] for g in "zrh"},
                **{f"linWb_{g}": pre["wts"][f"linWb_{g}"] for g in "zrh"},
            )
        )
    return in_maps


def _postprocess(res, pre):
    return res.results[0]["out"].astype(np.float32)


def kernel(**inputs) -> np.ndarray:
    pre = _preprocess(inputs)
    nc = _build(CB=pre["CB"])
    in_maps = _make_in_maps(pre)
    res = run_bass_kernel_spmd(nc, in_maps, core_ids=list(range(NCORE)))
    return _postprocess(res, pre)


if __name__ == "__main__":
    d = dict(np.load("/root/problem/inputs_cache.npz"))
    out = kernel(**d)
    print("kernel out:", out)


# revision 22
# speedup vs baseline: 5.3546x; 1.5472x over previous
"""TGCN (3-step GRU over GCN message passing) on 8 Trainium2 NeuronCores.

Strategy (dst-sharded message passing):
- Host relabels nodes (max-pool over nodes is permutation invariant) with a
  degree-balanced LPT assignment into 8 cores x 98 windows x 128 slots.
- Associativity: gcn(x@lin1) = (Anorm @ (dinv*x)) @ lin1 ... with lin1 and
  the conv weight folded into the gate projections on the host. The gather
  table is therefore dinv*x — pure host data, staged node-major in 4
  contiguous chunks of 25088 rows (int16-indexable). No phase A on device.
- Per (7-window group, src chunk): dma_gather (rotating over 4 SWDGE queues,
  with a deep descriptor-ring carveout so rings drain in parallel with
  generation) fetches per-edge source rows; 0/1 selection matrices built
  with iota+is_equal route each 128-edge block into the group's PSUM
  accumulator via the PE (scatter-add as matmul, gathered block stationary
  so the accumulator is feature-major - no transposes anywhere).
- Self-loops skip the gather: the feature-major dinv*x slice of the core's
  own nodes is DMA'd and added on the vector engine.
- GRU gates as 512-wide fp16 matmuls feature-major, conv+lin1 fused into
  the gate weights (biases are all zero); H stays resident in SBUF.
- Final: per-feature max over the core's nodes, AllReduce-max across cores,
  then the 128x10 output projection (identical on every core).
"""
import sys

sys.path.insert(0, "/opt/trn_rl_repo")

import numpy as np

import concourse.bass as bass
import concourse.mybir as mybir
import concourse.tile as tile
import concourse.bacc as bacc
from concourse.bass import broadcast_tensor_aps
from concourse.bass_utils import run_bass_kernel_spmd

F16 = mybir.dt.float16
F32 = mybir.dt.float32
I16 = mybir.dt.int16
I32 = mybir.dt.int32

N = 100000
E = 1600000
DIN = 128
DH = 128
DOUT = 10
P = 128
NCORE = 8
NW = 98               # windows (128-slot dst tiles) per core
SPC = NW * P          # 12544 slots per core
NSLOT = NCORE * SPC   # 100352
NT = NSLOT // P       # 784 global tiles
REAL_PC = 12500       # real nodes per core; pads at slots [12500, 12544)
CHN = 4               # source chunks: contiguous tile ranges of 196 tiles
CHTILES = NT // CHN   # 196
CHSZ = CHTILES * P    # 25088 rows per chunk (< 32768: int16-safe)
WGS = 7               # windows per gather group
NWG = NW // WGS       # 14 groups
TS = 3


def _preprocess(inputs):
    """Numpy-only host prep: node relabeling, edge sharding, input staging."""
    for b in ("lin1_b", "convb_z", "convb_r", "convb_h",
              "linb_z", "linb_r", "linb_h", "lin2_b"):
        assert np.abs(np.asarray(inputs[b])).max() == 0.0, f"{b} nonzero"

    import heapq

    edges = [np.asarray(inputs[f"edge{t}"]).astype(np.int64) for t in range(TS)]
    deg3 = np.zeros(N, np.int64)
    for t in range(TS):
        deg3 += np.bincount(edges[t][1], minlength=N)
    w_nodes = deg3 + 3

    order = np.argsort(-w_nodes, kind="stable")
    nbins = NCORE * NW
    cap = np.full(nbins, P, np.int32)
    cap[NW - 1 :: NW] = REAL_PC - (NW - 1) * P  # 84 real slots in last window
    heap = [(0, b) for b in range(nbins)]
    heapq.heapify(heap)
    bin_count = np.zeros(nbins, np.int32)
    bin_load = np.zeros(nbins, np.int64)
    assign_bin = np.empty(N, np.int32)
    slot_in_bin = np.empty(N, np.int32)
    for n in order:
        load, b = heapq.heappop(heap)
        assign_bin[n] = b
        slot_in_bin[n] = bin_count[b]
        bin_count[b] += 1
        bin_load[b] += w_nodes[n]
        if bin_count[b] < cap[b]:
            heapq.heappush(heap, (bin_load[b], b))
    core_of = assign_bin // NW
    w_of = assign_bin % NW
    gslot = (core_of * SPC + w_of * P + slot_in_bin).astype(np.int64)

    # degrees (with +1 self loop); pads get 1.0
    dinv = np.empty((TS, NSLOT), np.float32)
    for t in range(TS):
        dd = np.bincount(gslot[edges[t][1]], minlength=NSLOT).astype(np.float64)
        dd += 1.0  # self loops (pads harmlessly get deg 1: their rows are 0)
        dinv[t] = (1.0 / np.sqrt(dd)).astype(np.float32)

    # gather table: dinv * x, node-major rows in permuted slot order
    gtab = np.zeros((TS, NSLOT, DIN), np.float16)
    for t in range(TS):
        gtab[t, gslot] = (
            np.asarray(inputs[f"x{t}"]).astype(np.float32) * dinv[t, gslot][:, None]
        ).astype(np.float16)

    # feature-major per-core slice of the same table (self-loop add) in f32
    xselfT = np.empty((NCORE, TS, DIN, SPC), np.float32)
    for k in range(NCORE):
        sl = slice(k * SPC, (k + 1) * SPC)
        for t in range(TS):
            xselfT[k, t] = gtab[t, sl].astype(np.float32).T

    # dst-side dinv rows, replicated across partitions (DVE cannot
    # broadcast along the partition axis)
    dinv_myT = np.empty((NCORE, TS, 1, SPC), np.float32)
    for k in range(NCORE):
        dinv_myT[k, :, 0, :] = dinv[:, k * SPC : (k + 1) * SPC]
    dinv_myT = np.ascontiguousarray(np.broadcast_to(dinv_myT, (NCORE, TS, P, SPC)))

    # ---- edge cells: (core, window, chunk), capacity CB blocks each ----
    cellcnt_max = 0
    percell = []
    for t in range(TS):
        src, dst = edges[t]
        gs, gd = gslot[src], gslot[dst]
        key = (gd // P) * CHN + gs // CHSZ  # (core*NW + w) * CHN + chunk
        cnt = np.bincount(key, minlength=NCORE * NW * CHN)
        percell.append((gs, gd, key))
        cellcnt_max = max(cellcnt_max, int(cnt.max()))
    CB = max(5, -(-cellcnt_max // P))  # blocks per cell (uniform across cores)
    NBLK_W = CHN * CB                 # blocks per window
    CALL_BLK = WGS * CB               # blocks per (group, chunk) call
    NIDX_CALL = CALL_BLK * P
    CAPC = CB * P

    idx_arr = np.zeros((NCORE, TS, CHN, NWG, 16, NIDX_CALL // 16), np.int16)
    dst_arr = np.full((NCORE, TS, NWG, P, WGS * NBLK_W), -1.0, np.float16)

    ncell = NCORE * NW * CHN
    for t in range(TS):
        gs, gd, key = percell[t]
        srcloc = (gs % CHSZ).astype(np.int64)
        dstrel = gd % P
        o = np.argsort(key, kind="stable")
        key_s, srcloc_s, dstrel_s = key[o], srcloc[o], dstrel[o]
        cnt = np.bincount(key_s, minlength=ncell)
        starts = np.concatenate([[0], np.cumsum(cnt)[:-1]])
        rank = np.arange(len(key_s)) - starts[key_s]
        pad_src = np.zeros((ncell, CAPC), np.int64)
        pad_dst = np.full((ncell, CAPC), -1.0, np.float32)
        pad_src[key_s, rank] = srcloc_s
        pad_dst[key_s, rank] = dstrel_s
        pad_src = pad_src.reshape(NCORE, NW, CHN, CAPC)
        pad_dst = pad_dst.reshape(NCORE, NW, CHN, CAPC)
        for k in range(NCORE):
            for g in range(NWG):
                ws = slice(g * WGS, (g + 1) * WGS)
                for c in range(CHN):
                    flat = pad_src[k, ws, c].reshape(-1)  # [WGS*CAPC]
                    assert flat.max() < CHSZ
                    idx_arr[k, t, c, g] = (
                        flat.astype(np.int16).reshape(-1, 16).T
                    )
                # dst columns: (wl*NBLK_W + c*CB + b)
                d4 = pad_dst[k, ws].reshape(WGS, CHN * CB, P)
                dst_arr[k, t, g] = (
                    d4.reshape(WGS * NBLK_W, P).T.astype(np.float16)
                )

    idx_arr = np.ascontiguousarray(np.tile(idx_arr, (1, 1, 1, 1, 8, 1)))

    wts = dict(lin2_w=np.asarray(inputs["lin2_w"]).astype(np.float32))
    lin1 = np.asarray(inputs["lin1_w"]).astype(np.float32)
    for gname in "zrh":
        cw = np.asarray(inputs[f"convW_{gname}"]).astype(np.float32)
        lw = np.asarray(inputs[f"linW_{gname}"]).astype(np.float32)
        # fuse lin1 and conv into the gate projection: Z @ lin1 @ convW @ linW_top
        wts[f"Wg_{gname}"] = (lin1 @ cw @ lw[:DH]).astype(np.float16)
        wts[f"linWb_{gname}"] = lw[DH:].astype(np.float16)

    return dict(
        gtab=gtab, xselfT=xselfT, dinv_myT=dinv_myT,
        idx_arr=idx_arr, dst_arr=dst_arr, wts=wts, CB=CB,
    )


def _build(CB=5, ndev=NCORE):
    NBLK_W = CHN * CB
    CALL_BLK = WGS * CB
    NIDX_CALL = CALL_BLK * P
    ICOLS = NIDX_CALL // 16

    nc = bacc.Bacc("TRN2", target_bir_lowering=False, debug=False,
                   num_devices=ndev, num_swdge_queues=4,
                   dynamic_dma_scratch_size=32768)

    gtab_in = nc.dram_tensor("gtab", [TS, NSLOT, DIN], F16, kind="ExternalInput")
    xs_in = nc.dram_tensor("xselfT", [TS, DIN, SPC], F32, kind="ExternalInput")
    dim_in = nc.dram_tensor("dinv_myT", [TS, P, SPC], F32, kind="ExternalInput")
    idx_in = nc.dram_tensor("idx_arr", [TS, CHN, NWG, P, ICOLS], I16,
                            kind="ExternalInput")
    dst_in = nc.dram_tensor("dst_arr", [TS, NWG, P, WGS * NBLK_W], F16,
                            kind="ExternalInput")
    Wg_in = {g: nc.dram_tensor(f"Wg_{g}", [DIN, DH], F16, kind="ExternalInput")
             for g in "zrh"}
    linWb_in = {g: nc.dram_tensor(f"linWb_{g}", [DH, DH], F16, kind="ExternalInput")
                for g in "zrh"}
    lin2_in = nc.dram_tensor("lin2_w", [DH, DOUT], F32, kind="ExternalInput")
    out_t = nc.dram_tensor("out", [1, DOUT], F32, kind="ExternalOutput")

    with tile.TileContext(nc) as tc:
        with (
            tc.tile_pool(name="const", bufs=1) as cpool,
            tc.tile_pool(name="hpool", bufs=1) as hpool,
            tc.tile_pool(name="pa", bufs=2) as pa,
            tc.tile_pool(name="gb", bufs=2) as gb,          # gather bufs
            tc.tile_pool(name="bc", bufs=3) as bcp,         # phase B/C tiles
            tc.tile_pool(name="ps", bufs=1, space="PSUM") as ps,
            tc.tile_pool(name="dram", bufs=1, space="DRAM") as dr,
        ):
            # constants
            Wg_sb = {}
            linWb_sb = {}
            for g in "zrh":
                Wg_sb[g] = cpool.tile([DIN, DH], F16, tag=f"wg{g}", name=f"wg{g}")
                nc.sync.dma_start(Wg_sb[g][:], Wg_in[g][:])
                linWb_sb[g] = cpool.tile([DH, DH], F16, tag=f"lb{g}", name=f"lb{g}")
                nc.sync.dma_start(linWb_sb[g][:], linWb_in[g][:])
            lin2_sb = cpool.tile([DH, 16], F32, tag="l2")
            nc.gpsimd.memset(lin2_sb[:], 0.0)
            nc.sync.dma_start(lin2_sb[:, :DOUT], lin2_in[:])

            iota_i = cpool.tile([P, P], I32, tag="ioi")
            nc.gpsimd.iota(iota_i[:], pattern=[[1, P]], base=0, channel_multiplier=0)
            iota_f = cpool.tile([P, P], F16, tag="iof")
            nc.vector.tensor_copy(iota_f[:], iota_i[:])

            H_sb = hpool.tile([DH, SPC], F16, tag="H")
            nc.gpsimd.memset(H_sb[:], 0.0)

            for t in range(TS):
                for g in range(NWG):
                    Gt = [None] * CHN
                    for c in range(CHN):
                        ix = gb.tile([P, ICOLS], I16, tag=f"ix{c}")
                        nc.sync.dma_start(ix[:], idx_in[t, c, g])
                        Gt[c] = gb.tile([P, CALL_BLK * P], F16, tag=f"G{c}",
                                        name=f"G{c}")
                        g3 = Gt[c][:].rearrange("p (b q) -> p b q", q=P)
                        nc.gpsimd.dma_gather(
                            g3,
                            gtab_in[t, c * CHSZ : (c + 1) * CHSZ, :],
                            ix[:],
                            num_idxs=NIDX_CALL,
                            num_idxs_reg=NIDX_CALL,
                            elem_size=P,
                            single_packet=False,
                            queue_num=c,
                        )
                    dst_sb = gb.tile([P, WGS * NBLK_W], F16, tag="dst")
                    nc.sync.dma_start(dst_sb[:], dst_in[t, g])
                    dinv_g = pa.tile([P, WGS * P], F32, tag="dim", bufs=2)
                    nc.sync.dma_start(
                        dinv_g[:], dim_in[t, :, g * WGS * P : (g + 1) * WGS * P]
                    )

                    y_ps = [
                        ps.tile([P, 512], F32, tag="Y", name="Y0", bufs=4),
                        ps.tile([P, 512], F32, tag="Y", name="Y1", bufs=4),
                    ]
                    for wl in range(WGS):
                        w = g * WGS + wl
                        ycol = y_ps[wl // 4][:, (wl % 4) * P : (wl % 4 + 1) * P]
                        # selection matrices for this window's blocks in one op
                        M01 = bcp.tile([P, NBLK_W * P], F16, tag="m01")
                        m3 = M01[:].rearrange("p (b q) -> p b q", b=NBLK_W)
                        i0 = iota_f[:].rearrange("p (b q) -> p b q", b=1)
                        i1 = dst_sb[:, wl * NBLK_W : (wl + 1) * NBLK_W][:, :, None]
                        a0, a1 = broadcast_tensor_aps(i0, i1)
                        nc.vector.tensor_tensor(out=m3, in0=a0, in1=a1,
                                                op=mybir.AluOpType.is_equal)
                        # aggregate feature-major: Z^T[f, dst] += G^T M01
                        for c in range(CHN):
                            for b in range(CB):
                                nc.tensor.matmul(
                                    ycol,
                                    lhsT=Gt[c][:, (wl * CB + b) * P : (wl * CB + b + 1) * P],
                                    rhs=M01[:, (c * CB + b) * P : (c * CB + b + 1) * P],
                                    start=(c == 0 and b == 0),
                                    stop=(c == CHN - 1 and b == CB - 1),
                                )
                    # ---- GRU in 2 batches: windows [0:4) and [4:7) ----
                    for bi, (w0, nwb) in enumerate(((0, 4), (4, 3))):
                        W = nwb * P
                        n0 = (g * WGS + w0) * P  # node-column base
                        nsl = slice(n0, n0 + W)
                        Hsl = H_sb[:, nsl]
                        drow = dinv_g[:, w0 * P : w0 * P + W]
                        # self-loop rows (feature-major) + dst-side dinv
                        xself_sb = bcp.tile([P, 512], F32, tag="xself")
                        nc.sync.dma_start(xself_sb[:, :W], xs_in[t, :, nsl])
                        y0_sb = bcp.tile([P, 512], F32, tag="y0")
                        nc.vector.tensor_add(y0_sb[:, :W], y_ps[bi][:, :W],
                                             xself_sb[:, :W])
                        Zt_sb = bcp.tile([P, 512], F16, tag="Zt")
                        nc.vector.tensor_tensor(out=Zt_sb[:, :W], in0=y0_sb[:, :W],
                                                in1=drow, op=mybir.AluOpType.mult)
                        # gates (lin1+conv fused into Wg on host)
                        ZR = {}
                        for gname in "zr":
                            A_ps = ps.tile([P, 512], F32, tag="pc",
                                           name=f"Aps{gname}", bufs=4)
                            nc.tensor.matmul(A_ps[:, :W], lhsT=Wg_sb[gname][:],
                                             rhs=Zt_sb[:, :W], start=True, stop=False)
                            nc.tensor.matmul(A_ps[:, :W], lhsT=linWb_sb[gname][:],
                                             rhs=Hsl, start=False, stop=True)
                            ZR[gname] = bcp.tile([P, 512], F16, tag=gname.upper(),
                                                 name=gname.upper())
                            nc.scalar.activation(ZR[gname][:, :W], A_ps[:, :W],
                                                 mybir.ActivationFunctionType.Sigmoid)
                        HR = bcp.tile([P, 512], F16, tag="HR")
                        nc.vector.tensor_mul(HR[:, :W], Hsl, ZR["r"][:, :W])
                        A_ps = ps.tile([P, 512], F32, tag="pc", name="Apsh", bufs=4)
                        nc.tensor.matmul(A_ps[:, :W], lhsT=Wg_sb["h"][:],
                                         rhs=Zt_sb[:, :W], start=True, stop=False)
                        nc.tensor.matmul(A_ps[:, :W], lhsT=linWb_sb["h"][:],
                                         rhs=HR[:, :W], start=False, stop=True)
                        Ht = bcp.tile([P, 512], F16, tag="Ht")
                        nc.scalar.activation(Ht[:, :W], A_ps[:, :W],
                                             mybir.ActivationFunctionType.Tanh)
                        # H = Ht + Z*(H - Ht)
                        Hd = bcp.tile([P, 512], F16, tag="Hd")
                        nc.vector.tensor_sub(Hd[:, :W], Hsl, Ht[:, :W])
                        nc.vector.tensor_mul(Hd[:, :W], ZR["z"][:, :W], Hd[:, :W])
                        nc.vector.tensor_add(Hsl, Ht[:, :W], Hd[:, :W])

            # ---- final: masked max pool + AllReduce + projection ----
            nc.gpsimd.memset(H_sb[:, REAL_PC:SPC], -10000.0)
            hmax = cpool.tile([P, 1], F32, tag="hmax")
            nc.vector.reduce_max(hmax[:], H_sb[:], axis=mybir.AxisListType.X)
            cc_in = dr.tile([P, 1], F32)
            cc_out = dr.tile([P, 1], F32)
            nc.sync.dma_start(cc_in[:], hmax[:])
            if ndev > 1:
                nc.gpsimd.collective_compute(
                    "AllReduce",
                    mybir.AluOpType.max,
                    replica_groups=[list(range(NCORE))],
                    ins=[cc_in.opt()],
                    outs=[cc_out.opt()],
                )
            else:
                nc.gpsimd.dma_start(cc_out[:], cc_in[:])
            hg = cpool.tile([P, 1], F32, tag="hg")
            nc.sync.dma_start(hg[:], cc_out[:])
            o_ps = ps.tile([1, 16], F32, tag="pc", bufs=4)
            nc.tensor.matmul(o_ps[:, :16], lhsT=hg[:], rhs=lin2_sb[:],
                             start=True, stop=True)
            o_sb = cpool.tile([1, 16], F32, tag="osb")
            nc.vector.tensor_copy(o_sb[:], o_ps[:])
            nc.sync.dma_start(out_t[:], o_sb[:, :DOUT])

    nc.compile()
    return nc


def _make_in_maps(pre):
    in_maps = []
    for k in range(NCORE):
        in_maps.append(
            dict(
                gtab=pre["gtab"],
                xselfT=np.ascontiguousarray(pre["xselfT"][k]),
                dinv_myT=np.ascontiguousarray(pre["dinv_myT"][k]),
                idx_arr=np.ascontiguousarray(pre["idx_arr"][k]),
                dst_arr=np.ascontiguousarray(pre["dst_arr"][k]),
                lin2_w=pre["wts"]["lin2_w"],
                **{f"Wg_{g}": pre["wts"][f"Wg_{g}"] for g in "zrh"},
                **{f"linWb_{g}": pre["wts"][f"linWb_{g}"] for g in "zrh"},
            )
        )
    return in_maps


def _postprocess(res, pre):
    return res.results[0]["out"].astype(np.float32)


def kernel(**inputs) -> np.ndarray:
    pre = _preprocess(inputs)
    nc = _build(CB=pre["CB"])
    in_maps = _make_in_maps(pre)
    res = run_bass_kernel_spmd(nc, in_maps, core_ids=list(range(NCORE)))
    return _postprocess(res, pre)


if __name__ == "__main__":
    d = dict(np.load("/root/problem/inputs_cache.npz"))
    out = kernel(**d)
    print("kernel out:", out)
